# revision 5
# baseline (speedup 1.0000x reference)
"""CIF (continuous integrate-and-fire) segment-reduce kernel for Trainium2.

Strategy
--------
The CIF recurrence over T is sequential only in the *scalar* alpha stream
(B*T = 64K f32 values).  The heavy part - accumulating alpha-weighted hidden
vectors into label slots - is a banded matmul  out[b] = W_b @ hidden[b]
with W_b in R^{L x T} holding at most 2 nonzeros per column:

  * timestep t contributes weight cur_t to the slot of the next fire at-or-
    after t (slotA), and
  * weight rem_t to the slot of the next fire strictly after t (slotB =
    slotA+1, nonzero only at fire steps).

The host replicates the reference's f32 scan bit-exactly (same IEEE ops in
the same order) to derive (slotA, slotB, wA, wB) per timestep, then expands
them into dense per-chunk weight tiles W[c][t, row] (row = slot mod 128 on
the fast path) and packs both W and hidden into partition-major bf16
layouts.  The device is then a pure DMA + matmul pipeline: per 128-timestep
chunk one bf16 LDWEIGHTS+MATMUL accumulates into a 128-slot PSUM window
(bf16 matmul runs at 1 cycle/row vs fp32's 4, and bf16 halves the HBM
traffic, which is the roofline here).

Because alphas rows sum to exactly L, slot(t) tracks 0.125*t with only a
few slots of drift, so chunks grouped 4-at-a-time land in a static 128-slot
PSUM window per group (bases 0/32/96/128, PSUM row = slot mod 128); the
overlapping windows are combined into the [256, H] output with a handful of
PSUM->SBUF copies/adds, downcast to bf16, and written back (host upcasts).
The host verifies every contribution fits its window and falls back to a
generic full-width (two matmuls per chunk) program otherwise.

DMA: only SP(sync) and Activation(scalar) have hardware DGE queues, so the
~10.4MB/core of reads is split across both: hidden groups 0-1 + all W on
sync, hidden groups 2-3 + output writes on scalar.  All loads are issued
up front (everything fits SBUF) so both queues stream back-to-back.

Sharding: pure data parallelism - batch 32 is split 4-per-core across the
8 NeuronCores; no communication.
"""

import sys

if "/opt/trn_rl_repo" not in sys.path:
    sys.path.insert(0, "/opt/trn_rl_repo")

import ml_dtypes
import numpy as np

import concourse.tile as tile
from concourse import bacc, mybir
from concourse.bass_utils import run_bass_kernel_spmd

# Problem constants (hardcoded per the task contract).
B, T, H, L = 32, 2048, 512, 256
N_CORES = 8
B_PER_CORE = B // N_CORES          # 4
TCHUNK = 128                       # timesteps per matmul contraction chunk
NCHUNK = T // TCHUNK               # 16
GROUP = 4                          # chunks per PSUM window group
NGROUP = NCHUNK // GROUP           # 4
WIN_BASE = (0, 32, 96, 128)        # static slot-window base per group
F32 = mybir.dt.float32
BF16 = mybir.dt.bfloat16
NPBF16 = ml_dtypes.bfloat16

_compiled = {}  # variant -> (nc, out_name)


def host_scan(alphas: np.ndarray) -> tuple[np.ndarray, ...]:
    """Replicate the reference's sequential f32 scan exactly.

    Returns slotA, slotB (int label indices) and wA, wB (f32 weights),
    each [B, T]:  out[b, l] = sum_t (slotA==l)*wA*h_t + (slotB==l)*wB*h_t.
    """
    Bn, Tn = alphas.shape
    one = np.float32(1.0)
    thr = np.float32(0.95)
    integrate = np.zeros(Bn, np.float32)
    fire_all = np.zeros((Bn, Tn), bool)
    cur_all = np.empty((Bn, Tn), np.float32)
    rem_all = np.empty((Bn, Tn), np.float32)
    for t in range(Tn):
        at = alphas[:, t]
        dist = one - integrate
        integrate = integrate + at
        fire = integrate > thr
        integrate = np.where(fire, integrate - one, integrate)
        cur = np.where(fire, dist, at)
        fire_all[:, t] = fire
        cur_all[:, t] = cur
        rem_all[:, t] = at - cur

    k_t = np.cumsum(fire_all, axis=1)        # fires up to and including t
    n_before = k_t - fire_all                # fires strictly before t
    total = k_t[:, -1:]
    slotA = np.minimum(n_before, L - 1).astype(np.int64)
    slotB = np.minimum(k_t, L - 1).astype(np.int64)
    wA = np.where(n_before < total, cur_all, np.float32(0.0))
    wB = np.where(k_t < total, rem_all, np.float32(0.0))
    return slotA, slotB, wA, wB


def _window_ok(slotA, slotB, wA, wB) -> bool:
    """Every nonzero contribution must land inside its chunk-group window."""
    for g in range(NGROUP):
        base = WIN_BASE[g]
        sl = slice(g * GROUP * TCHUNK, (g + 1) * GROUP * TCHUNK)
        for s, w in ((slotA[:, sl], wA[:, sl]), (slotB[:, sl], wB[:, sl])):
            m = w != 0
            if m.any():
                v = s[m]
                if v.min() < base or v.max() > base + 127:
                    return False
    return True


def _expand_w(slotA, slotB, wA, wB, nrows: int) -> np.ndarray:
    """Dense weight tiles [B, 128(t within chunk), NCHUNK*nrows] bf16.

    Column c*nrows + r of partition p holds the weight of timestep
    t = c*128 + p into PSUM row r (row = slot mod nrows)."""
    rA = (slotA % nrows).astype(np.int64)
    rB = (slotB % nrows).astype(np.int64)
    w = np.zeros((B, T, nrows), np.float32)
    bt = np.arange(B * T)
    np.add.at(w.reshape(-1, nrows), (bt, rA.reshape(-1)), wA.reshape(-1))
    np.add.at(w.reshape(-1, nrows), (bt, rB.reshape(-1)), wB.reshape(-1))
    # [B, T, nrows] -> [B, NCHUNK, 128, nrows] -> [B, 128, NCHUNK, nrows]
    w = w.reshape(B, NCHUNK, TCHUNK, nrows).transpose(0, 2, 1, 3)
    return np.ascontiguousarray(w.reshape(B, TCHUNK, NCHUNK * nrows)).astype(NPBF16)


def _pack_hidden(hidden: np.ndarray) -> np.ndarray:
    """[B, 128(t within chunk), NCHUNK*H] bf16: col c*H + h of partition p
    holds hidden[b, c*128 + p, h]."""
    hp = hidden.reshape(B, NCHUNK, TCHUNK, H).transpose(0, 2, 1, 3)
    return np.ascontiguousarray(hp.reshape(B, TCHUNK, NCHUNK * H)).astype(NPBF16)


def build_program_windowed():
    """Fast path: one bf16 matmul per chunk into a static 128-slot window."""
    nc = bacc.Bacc("TRN2", target_bir_lowering=False, debug=False)

    hid_d = nc.dram_tensor("hidp", [B_PER_CORE, TCHUNK, NCHUNK * H], BF16,
                           kind="ExternalInput")
    w_d = nc.dram_tensor("wp", [B_PER_CORE, TCHUNK, NCHUNK * TCHUNK], BF16,
                         kind="ExternalInput")
    out_d = nc.dram_tensor("out", [B_PER_CORE, L, H], BF16, kind="ExternalOutput")

    GW = GROUP * TCHUNK   # w cols per group (512)
    GH = GROUP * H        # hidden cols per group (2048)

    with tile.TileContext(nc) as tc:
        with (
            tc.tile_pool(name="hid", bufs=16) as hidp,
            tc.tile_pool(name="wts", bufs=4) as wpool,
            tc.tile_pool(name="outp", bufs=2) as outp,
            tc.tile_pool(name="psum", bufs=8, space="PSUM") as psump,
        ):
            # Issue every load up front (10.4MB fits SBUF).  The two HWDGE
            # queues share ~380GB/s of per-core HBM bandwidth, so balance
            # bytes: sync gets hidden groups 0-1 + W for batch 0, scalar
            # gets hidden groups 2-3 + W for batch 1 (4.69MB each); W for
            # the late batches 2-3 rides the gpsimd software-DGE queue as a
            # third lane (it is slow, but those aren't needed until late).
            # Output writes alternate between the HW queues.
            hts, wts = {}, {}
            for i in range(B_PER_CORE):
                wt = wpool.tile([TCHUNK, NCHUNK * TCHUNK], BF16)
                if i < 2:
                    weng = nc.sync if i == 0 else nc.scalar
                    weng.dma_start(wt[:], w_d[i])
                else:
                    # software DGE: split per group so consumption can
                    # start before the whole 512KB lands
                    for g in range(NGROUP):
                        nc.gpsimd.dma_start(wt[:, g * GW:(g + 1) * GW],
                                            w_d[i, :, g * GW:(g + 1) * GW])
                wts[i] = wt
                for g in range(NGROUP):
                    ht = hidp.tile([TCHUNK, GH], BF16)
                    src = hid_d[i, :, g * GH:(g + 1) * GH]
                    eng = nc.sync if g < 2 else nc.scalar
                    if i == 0:
                        # split the first loads per-chunk so the first
                        # matmul starts after 128KB instead of 512KB
                        for cc in range(GROUP):
                            eng.dma_start(ht[:, cc * H:(cc + 1) * H],
                                          hid_d[i, :, g * GH + cc * H:
                                                g * GH + (cc + 1) * H])
                    else:
                        eng.dma_start(ht[:], src)
                    hts[i, g] = ht

            for i in range(B_PER_CORE):
                wt = wts[i]
                ob = outp.tile([TCHUNK, 2, H], BF16, tag="ob")
                oeng = nc.scalar if i % 2 == 0 else nc.sync
                ps = []
                for g in range(NGROUP):
                    ht = hts[i, g]
                    psg = psump.tile([TCHUNK, H], F32)
                    ps.append(psg)
                    for cc in range(GROUP):
                        c = g * GROUP + cc
                        nc.tensor.matmul(
                            psg[:], wt[:, c * TCHUNK:(c + 1) * TCHUNK],
                            ht[:, cc * H:(cc + 1) * H],
                            start=(cc == 0), stop=(cc == GROUP - 1),
                        )
                    # Combine overlapping windows as soon as deps allow
                    # (slot coverage: g0 0..127, g1 32..159, g2 96..223,
                    # g3 128..255; PSUM row = slot mod 128 so every slice
                    # is partition-aligned).  The ACT copy downcasts into
                    # the bf16 out tile; DVE then accumulates the overlap
                    # rows in place, so every add has exactly one PSUM
                    # operand and no staging tiles are needed (HW rules:
                    # one PSUM input per tensor_tensor; partition ranges
                    # starting at 32 span max 32, at 64 max 64).
                    if g == 2:
                        nc.scalar.copy(ob[:, 0, :], ps[0][:])
                        nc.vector.tensor_add(
                            ob[32:64, 0, :], ob[32:64, 0, :], ps[1][32:64, :]
                        )
                        nc.vector.tensor_add(
                            ob[64:128, 0, :], ob[64:128, 0, :], ps[1][64:128, :]
                        )
                        nc.vector.tensor_add(
                            ob[96:128, 0, :], ob[96:128, 0, :], ps[2][96:128, :]
                        )
                        oeng.dma_start(out_d[i, 0:128, :], ob[:, 0, :])
                nc.scalar.copy(ob[:, 1, :], ps[3][:])
                nc.vector.tensor_add(
                    ob[0:96, 1, :], ob[0:96, 1, :], ps[2][0:96, :]
                )
                nc.vector.tensor_add(
                    ob[0:32, 1, :], ob[0:32, 1, :], ps[1][0:32, :]
                )
                oeng.dma_start(out_d[i, 128:256, :], ob[:, 1, :])

    nc.compile()
    return nc, out_d.name


def build_program_generic():
    """Fallback: full-width weights, two matmuls per chunk."""
    nc = bacc.Bacc("TRN2", target_bir_lowering=False, debug=False)

    hid_d = nc.dram_tensor("hidp", [B_PER_CORE, TCHUNK, NCHUNK * H], BF16,
                           kind="ExternalInput")
    w_d = nc.dram_tensor("wp", [B_PER_CORE, TCHUNK, NCHUNK * L], BF16,
                         kind="ExternalInput")
    out_d = nc.dram_tensor("out", [B_PER_CORE, L, H], BF16, kind="ExternalOutput")

    with tile.TileContext(nc) as tc:
        with (
            tc.tile_pool(name="hid", bufs=4) as hidp,
            tc.tile_pool(name="wts", bufs=4) as wpool,
            tc.tile_pool(name="outp", bufs=2) as outp,
            tc.tile_pool(name="psum", bufs=4, space="PSUM") as psump,
        ):
            for i in range(B_PER_CORE):
                ps0 = psump.tile([TCHUNK, H], F32)
                ps1 = psump.tile([TCHUNK, H], F32)
                for c in range(NCHUNK):
                    ht = hidp.tile([TCHUNK, H], BF16)
                    nc.sync.dma_start(ht[:], hid_d[i, :, c * H:(c + 1) * H])
                    wt = wpool.tile([TCHUNK, L], BF16)
                    nc.scalar.dma_start(wt[:], w_d[i, :, c * L:(c + 1) * L])
                    nc.tensor.matmul(
                        ps0[:], wt[:, 0:128], ht[:],
                        start=(c == 0), stop=(c == NCHUNK - 1),
                    )
                    nc.tensor.matmul(
                        ps1[:], wt[:, 128:256], ht[:],
                        start=(c == 0), stop=(c == NCHUNK - 1),
                    )
                o0 = outp.tile([128, H], BF16, tag="o0")
                nc.scalar.copy(o0[:], ps0[:])
                o1 = outp.tile([128, H], BF16, tag="o1")
                nc.scalar.copy(o1[:], ps1[:])
                nc.sync.dma_start(out_d[i, 0:128, :], o0[:])
                nc.sync.dma_start(out_d[i, 128:256, :], o1[:])

    nc.compile()
    return nc, out_d.name


def _get_compiled(variant: str):
    if variant not in _compiled:
        _compiled[variant] = (
            build_program_windowed() if variant == "windowed"
            else build_program_generic()
        )
    return _compiled[variant]


def prepare(hidden: np.ndarray, alphas: np.ndarray):
    """Host scan + input packing. Returns (variant, in_maps)."""
    slotA, slotB, wA, wB = host_scan(alphas)
    if _window_ok(slotA, slotB, wA, wB):
        variant = "windowed"
        w = _expand_w(slotA, slotB, wA, wB, TCHUNK)
    else:
        variant = "generic"
        w = _expand_w(slotA, slotB, wA, wB, L)
    hidp = _pack_hidden(hidden)
    in_maps = []
    for j in range(N_CORES):
        sl = slice(j * B_PER_CORE, (j + 1) * B_PER_CORE)
        in_maps.append({"hidp": hidp[sl], "wp": w[sl]})
    return variant, in_maps


def run_sharded(hidden: np.ndarray, alphas: np.ndarray, trace: bool = False, **kw):
    """Run the SPMD kernel; returns (out [B,L,H] f32, BassKernelResults)."""
    variant, in_maps = prepare(hidden, alphas)
    nc, out_name = _get_compiled(variant)
    res = run_bass_kernel_spmd(nc, in_maps, list(range(N_CORES)), trace=trace, **kw)
    out = np.concatenate([r[out_name] for r in res.results], axis=0)
    return out.astype(np.float32), res


def kernel(hidden, alphas, num_labels=L) -> np.ndarray:
    hidden = np.asarray(hidden, dtype=np.float32)
    alphas = np.asarray(alphas, dtype=np.float32)
    assert hidden.shape == (B, T, H) and alphas.shape == (B, T)
    assert int(num_labels) == L
    out, _ = run_sharded(hidden, alphas)
    return out


# revision 14
# speedup vs baseline: 1.1650x; 1.1650x over previous
"""CIF (continuous integrate-and-fire) segment-reduce kernel for Trainium2.

Strategy
--------
The CIF recurrence over T is sequential only in the *scalar* alpha stream
(B*T = 64K f32 values).  The heavy part - accumulating alpha-weighted hidden
vectors into label slots - is a banded matmul  out[b] = W_b @ hidden[b]
with W_b in R^{L x T} holding at most 2 nonzeros per column:

  * timestep t contributes weight cur_t to the slot of the next fire at-or-
    after t (slotA), and
  * weight rem_t to the slot of the next fire strictly after t (slotB =
    slotA+1, nonzero only at fire steps).

The host replicates the reference's f32 scan bit-exactly (same IEEE ops in
the same order) to derive (slotA, slotB, wA, wB) per timestep, then expands
them into dense per-chunk weight tiles W[c][t, row] (row = slot mod 128 on
the fast path) and packs both W and hidden into partition-major bf16
layouts.  The device is then a pure DMA + matmul pipeline: per 128-timestep
chunk one bf16 LDWEIGHTS+MATMUL accumulates into a 128-slot PSUM window
(bf16 matmul runs at 1 cycle/row vs fp32's 4, and bf16 halves the HBM
traffic, which is the roofline here).

Because alphas rows sum to exactly L, slot(t) tracks 0.125*t with only a
few slots of drift, so chunks grouped 4-at-a-time land in a static 128-slot
PSUM window per group (bases 0/32/96/128, PSUM row = slot mod 128); the
overlapping windows are combined into the [256, H] output with a handful of
PSUM->SBUF copies/adds, downcast to bf16, and written back (host upcasts).
The host verifies every contribution fits its window and falls back to a
generic full-width (two matmuls per chunk) program otherwise.

DMA: only SP(sync) and Activation(scalar) have hardware DGE queues, so the
~10.4MB/core of reads is split across both: hidden groups 0-1 + all W on
sync, hidden groups 2-3 + output writes on scalar.  All loads are issued
up front (everything fits SBUF) so both queues stream back-to-back.

Sharding: pure data parallelism - batch 32 is split 4-per-core across the
8 NeuronCores; no communication.
"""

import sys

if "/opt/trn_rl_repo" not in sys.path:
    sys.path.insert(0, "/opt/trn_rl_repo")

import ml_dtypes
import numpy as np

import concourse.tile as tile
from concourse import bacc, mybir
from concourse.bass_utils import run_bass_kernel_spmd

# Problem constants (hardcoded per the task contract).
B, T, H, L = 32, 2048, 512, 256
N_CORES = 8
B_PER_CORE = B // N_CORES          # 4
TCHUNK = 128                       # timesteps per matmul contraction chunk
NCHUNK = T // TCHUNK               # 16
GROUP = 4                          # chunks per PSUM window group
NGROUP = NCHUNK // GROUP           # 4
WIN_BASE = (0, 32, 96, 128)        # static slot-window base per group
F32 = mybir.dt.float32
BF16 = mybir.dt.bfloat16
NPBF16 = ml_dtypes.bfloat16

_compiled = {}  # variant -> (nc, out_name)


def host_scan(alphas: np.ndarray) -> tuple[np.ndarray, ...]:
    """Replicate the reference's sequential f32 scan exactly.

    Returns slotA, slotB (int label indices) and wA, wB (f32 weights),
    each [B, T]:  out[b, l] = sum_t (slotA==l)*wA*h_t + (slotB==l)*wB*h_t.
    """
    Bn, Tn = alphas.shape
    one = np.float32(1.0)
    thr = np.float32(0.95)
    integrate = np.zeros(Bn, np.float32)
    fire_all = np.zeros((Bn, Tn), bool)
    cur_all = np.empty((Bn, Tn), np.float32)
    rem_all = np.empty((Bn, Tn), np.float32)
    for t in range(Tn):
        at = alphas[:, t]
        dist = one - integrate
        integrate = integrate + at
        fire = integrate > thr
        integrate = np.where(fire, integrate - one, integrate)
        cur = np.where(fire, dist, at)
        fire_all[:, t] = fire
        cur_all[:, t] = cur
        rem_all[:, t] = at - cur

    k_t = np.cumsum(fire_all, axis=1)        # fires up to and including t
    n_before = k_t - fire_all                # fires strictly before t
    total = k_t[:, -1:]
    slotA = np.minimum(n_before, L - 1).astype(np.int64)
    slotB = np.minimum(k_t, L - 1).astype(np.int64)
    wA = np.where(n_before < total, cur_all, np.float32(0.0))
    wB = np.where(k_t < total, rem_all, np.float32(0.0))
    return slotA, slotB, wA, wB


def _window_ok(slotA, slotB, wA, wB) -> bool:
    """Every nonzero contribution must land inside its chunk-group window."""
    for g in range(NGROUP):
        base = WIN_BASE[g]
        sl = slice(g * GROUP * TCHUNK, (g + 1) * GROUP * TCHUNK)
        for s, w in ((slotA[:, sl], wA[:, sl]), (slotB[:, sl], wB[:, sl])):
            m = w != 0
            if m.any():
                v = s[m]
                if v.min() < base or v.max() > base + 127:
                    return False
    return True


def _expand_w(slotA, slotB, wA, wB, nrows: int) -> np.ndarray:
    """Dense weight tiles [B, 128(t within chunk), NCHUNK*nrows] bf16.

    Column c*nrows + r of partition p holds the weight of timestep
    t = c*128 + p into PSUM row r (row = slot mod nrows)."""
    rA = (slotA % nrows).astype(np.int64)
    rB = (slotB % nrows).astype(np.int64)
    w = np.zeros((B, T, nrows), np.float32)
    bt = np.arange(B * T)
    np.add.at(w.reshape(-1, nrows), (bt, rA.reshape(-1)), wA.reshape(-1))
    np.add.at(w.reshape(-1, nrows), (bt, rB.reshape(-1)), wB.reshape(-1))
    # [B, T, nrows] -> [B, NCHUNK, 128, nrows] -> [B, 128, NCHUNK, nrows]
    w = w.reshape(B, NCHUNK, TCHUNK, nrows).transpose(0, 2, 1, 3)
    return np.ascontiguousarray(w.reshape(B, TCHUNK, NCHUNK * nrows)).astype(NPBF16)


def _pack_hidden(hidden: np.ndarray) -> np.ndarray:
    """[B, 128(t within chunk), NCHUNK*H] bf16: col c*H + h of partition p
    holds hidden[b, c*128 + p, h]."""
    hp = hidden.reshape(B, NCHUNK, TCHUNK, H).transpose(0, 2, 1, 3)
    return np.ascontiguousarray(hp.reshape(B, TCHUNK, NCHUNK * H)).astype(NPBF16)


def build_program_windowed():
    """Fast path: one bf16 matmul per chunk into a static 128-slot window."""
    nc = bacc.Bacc("TRN2", target_bir_lowering=False, debug=False,
                   enable_partition_id=False)

    hid_d = nc.dram_tensor("hidp", [B_PER_CORE, TCHUNK, NCHUNK * H], BF16,
                           kind="ExternalInput")
    w_d = nc.dram_tensor("wp", [B_PER_CORE, TCHUNK, NCHUNK * TCHUNK], BF16,
                         kind="ExternalInput")
    out_d = nc.dram_tensor("out", [B_PER_CORE, L, H], BF16, kind="ExternalOutput")

    GW = GROUP * TCHUNK   # w cols per group (512)
    GH = GROUP * H        # hidden cols per group (2048)

    with tile.TileContext(nc) as tc:
        with (
            tc.tile_pool(name="hid", bufs=16) as hidp,
            tc.tile_pool(name="wts", bufs=4) as wpool,
            tc.tile_pool(name="outp", bufs=2) as outp,
            tc.tile_pool(name="psum", bufs=8, space="PSUM") as psump,
        ):
            # Issue every load up front (10.4MB fits SBUF).  The two HWDGE
            # queues share ~380GB/s of per-core HBM bandwidth (a gpsimd
            # software-DGE third lane just steals from the same pool and
            # measures slower), so balance bytes: sync gets hidden groups
            # 0-1 + W for even batches (5.19MB), scalar gets hidden groups
            # 2-3 + W for odd batches (5.19MB); output writes alternate.
            hts, wts = {}, {}
            for i in range(B_PER_CORE):
                wt = wpool.tile([TCHUNK, NCHUNK * TCHUNK], BF16)
                weng = nc.sync if i % 2 == 0 else nc.scalar
                weng.dma_start(wt[:], w_d[i])
                wts[i] = wt
                for g in range(NGROUP):
                    ht = hidp.tile([TCHUNK, GH], BF16)
                    src = hid_d[i, :, g * GH:(g + 1) * GH]
                    eng = nc.sync if g < 2 else nc.scalar
                    if i == 0 and g in (0, 2):
                        # halve the queue-head loads so the first matmuls
                        # start earlier; finer splits shrink the descriptor
                        # size (1KB/partition) and measurably sag the HBM
                        # stream, so keep everything else at 4KB/partition
                        h2 = GH // 2
                        eng.dma_start(ht[:, 0:h2],
                                      hid_d[i, :, g * GH:g * GH + h2])
                        eng.dma_start(ht[:, h2:GH],
                                      hid_d[i, :, g * GH + h2:(g + 1) * GH])
                    else:
                        eng.dma_start(ht[:], src)
                    hts[i, g] = ht

            for i in range(B_PER_CORE):
                wt = wts[i]
                # separate tiles per output half: a DMA read waits on ALL
                # writers of its tile, so a shared [128, 2, H] tile made the
                # half-0 write falsely depend on half-1's combine
                ob0 = outp.tile([TCHUNK, H], BF16, tag="ob0")
                ob1 = outp.tile([TCHUNK, H], BF16, tag="ob1")
                ps = []
                for g in range(NGROUP):
                    ht = hts[i, g]
                    psg = psump.tile([TCHUNK, H], F32)
                    ps.append(psg)
                    for cc in range(GROUP):
                        c = g * GROUP + cc
                        nc.tensor.matmul(
                            psg[:], wt[:, c * TCHUNK:(c + 1) * TCHUNK],
                            ht[:, cc * H:(cc + 1) * H],
                            start=(cc == 0), stop=(cc == GROUP - 1),
                        )
                    # Combine overlapping windows as soon as deps allow
                    # (slot coverage: g0 0..127, g1 32..159, g2 96..223,
                    # g3 128..255; PSUM row = slot mod 128 so every slice
                    # is partition-aligned).  The ACT copy downcasts into
                    # the bf16 out tile; DVE accumulates overlap rows in
                    # place, so every add has exactly one PSUM operand and
                    # no staging tiles are needed (HW rules: one PSUM input
                    # per tensor_tensor; partition ranges starting at 32
                    # span max 32, at 64 max 64).
                    if g == 2:
                        nc.scalar.copy(ob0[:], ps[0][:])
                        nc.vector.tensor_add(
                            ob0[32:64, :], ob0[32:64, :], ps[1][32:64, :]
                        )
                        nc.vector.tensor_add(
                            ob0[64:128, :], ob0[64:128, :], ps[1][64:128, :]
                        )
                        nc.vector.tensor_add(
                            ob0[96:128, :], ob0[96:128, :], ps[2][96:128, :]
                        )
                        nc.gpsimd.dma_start(out_d[i, 0:128, :], ob0[:])
                nc.scalar.copy(ob1[:], ps[3][:])
                nc.vector.tensor_add(ob1[0:96, :], ob1[0:96, :], ps[2][0:96, :])
                nc.vector.tensor_add(ob1[0:32, :], ob1[0:32, :], ps[1][0:32, :])
                nc.gpsimd.dma_start(out_d[i, 128:256, :], ob1[:])

    nc.compile()
    return nc, out_d.name


def build_program_generic():
    """Fallback: full-width weights, two matmuls per chunk."""
    nc = bacc.Bacc("TRN2", target_bir_lowering=False, debug=False)

    hid_d = nc.dram_tensor("hidp", [B_PER_CORE, TCHUNK, NCHUNK * H], BF16,
                           kind="ExternalInput")
    w_d = nc.dram_tensor("wp", [B_PER_CORE, TCHUNK, NCHUNK * L], BF16,
                         kind="ExternalInput")
    out_d = nc.dram_tensor("out", [B_PER_CORE, L, H], BF16, kind="ExternalOutput")

    with tile.TileContext(nc) as tc:
        with (
            tc.tile_pool(name="hid", bufs=4) as hidp,
            tc.tile_pool(name="wts", bufs=4) as wpool,
            tc.tile_pool(name="outp", bufs=2) as outp,
            tc.tile_pool(name="psum", bufs=4, space="PSUM") as psump,
        ):
            for i in range(B_PER_CORE):
                ps0 = psump.tile([TCHUNK, H], F32)
                ps1 = psump.tile([TCHUNK, H], F32)
                for c in range(NCHUNK):
                    ht = hidp.tile([TCHUNK, H], BF16)
                    nc.sync.dma_start(ht[:], hid_d[i, :, c * H:(c + 1) * H])
                    wt = wpool.tile([TCHUNK, L], BF16)
                    nc.scalar.dma_start(wt[:], w_d[i, :, c * L:(c + 1) * L])
                    nc.tensor.matmul(
                        ps0[:], wt[:, 0:128], ht[:],
                        start=(c == 0), stop=(c == NCHUNK - 1),
                    )
                    nc.tensor.matmul(
                        ps1[:], wt[:, 128:256], ht[:],
                        start=(c == 0), stop=(c == NCHUNK - 1),
                    )
                o0 = outp.tile([128, H], BF16, tag="o0")
                nc.scalar.copy(o0[:], ps0[:])
                o1 = outp.tile([128, H], BF16, tag="o1")
                nc.scalar.copy(o1[:], ps1[:])
                nc.sync.dma_start(out_d[i, 0:128, :], o0[:])
                nc.sync.dma_start(out_d[i, 128:256, :], o1[:])

    nc.compile()
    return nc, out_d.name


def _get_compiled(variant: str):
    if variant not in _compiled:
        _compiled[variant] = (
            build_program_windowed() if variant == "windowed"
            else build_program_generic()
        )
    return _compiled[variant]


def prepare(hidden: np.ndarray, alphas: np.ndarray):
    """Host scan + input packing. Returns (variant, in_maps)."""
    slotA, slotB, wA, wB = host_scan(alphas)
    if _window_ok(slotA, slotB, wA, wB):
        variant = "windowed"
        w = _expand_w(slotA, slotB, wA, wB, TCHUNK)
    else:
        variant = "generic"
        w = _expand_w(slotA, slotB, wA, wB, L)
    hidp = _pack_hidden(hidden)
    in_maps = []
    for j in range(N_CORES):
        sl = slice(j * B_PER_CORE, (j + 1) * B_PER_CORE)
        in_maps.append({"hidp": hidp[sl], "wp": w[sl]})
    return variant, in_maps


def run_sharded(hidden: np.ndarray, alphas: np.ndarray, trace: bool = False, **kw):
    """Run the SPMD kernel; returns (out [B,L,H] f32, BassKernelResults)."""
    variant, in_maps = prepare(hidden, alphas)
    nc, out_name = _get_compiled(variant)
    res = run_bass_kernel_spmd(nc, in_maps, list(range(N_CORES)), trace=trace, **kw)
    out = np.concatenate([r[out_name] for r in res.results], axis=0)
    return out.astype(np.float32), res


def kernel(hidden, alphas, num_labels=L) -> np.ndarray:
    hidden = np.asarray(hidden, dtype=np.float32)
    alphas = np.asarray(alphas, dtype=np.float32)
    assert hidden.shape == (B, T, H) and alphas.shape == (B, T)
    assert int(num_labels) == L
    out, _ = run_sharded(hidden, alphas)
    return out


# revision 22
# speedup vs baseline: 1.1698x; 1.0041x over previous
"""CIF (continuous integrate-and-fire) segment-reduce kernel for Trainium2.

Strategy
--------
The CIF recurrence over T is sequential only in the *scalar* alpha stream
(B*T = 64K f32 values).  The heavy part - accumulating alpha-weighted hidden
vectors into label slots - is a banded matmul  out[b] = W_b @ hidden[b]
with W_b in R^{L x T} holding at most 2 nonzeros per column:

  * timestep t contributes weight cur_t to the slot of the next fire at-or-
    after t (slotA), and
  * weight rem_t to the slot of the next fire strictly after t (slotB =
    slotA+1, nonzero only at fire steps).

The host replicates the reference's f32 scan bit-exactly (same IEEE ops in
the same order) to derive (slotA, slotB, wA, wB) per timestep, then expands
them into dense per-chunk weight tiles W[c][t, row] (row = slot mod 128 on
the fast path) and packs both W and hidden into partition-major bf16
layouts.  The device is then a pure DMA + matmul pipeline: per 128-timestep
chunk one bf16 LDWEIGHTS+MATMUL accumulates into a 128-slot PSUM window
(bf16 matmul runs at 1 cycle/row vs fp32's 4, and bf16 halves the HBM
traffic, which is the roofline here).

Because alphas rows sum to exactly L, slot(t) tracks 0.125*t with only a
few slots of drift, so chunks grouped 4-at-a-time land in a static 128-slot
PSUM window per group (bases 0/32/96/128, PSUM row = slot mod 128); the
overlapping windows are combined into the [256, H] output with a handful of
PSUM->SBUF copies/adds, downcast to bf16, and written back (host upcasts).
The host verifies every contribution fits its window and falls back to a
generic full-width (two matmuls per chunk) program otherwise.

DMA: only SP(sync) and Activation(scalar) have hardware DGE queues, so the
~10.4MB/core of reads is split across both: hidden groups 0-1 + all W on
sync, hidden groups 2-3 + output writes on scalar.  All loads are issued
up front (everything fits SBUF) so both queues stream back-to-back.

Sharding: pure data parallelism - batch 32 is split 4-per-core across the
8 NeuronCores; no communication.
"""

import sys

if "/opt/trn_rl_repo" not in sys.path:
    sys.path.insert(0, "/opt/trn_rl_repo")

import ml_dtypes
import numpy as np

import concourse.tile as tile
from concourse import bacc, mybir
from concourse.bass_utils import run_bass_kernel_spmd

# Problem constants (hardcoded per the task contract).
B, T, H, L = 32, 2048, 512, 256
N_CORES = 8
B_PER_CORE = B // N_CORES          # 4
TCHUNK = 128                       # timesteps per matmul contraction chunk
NCHUNK = T // TCHUNK               # 16
GROUP = 4                          # chunks per PSUM window group
NGROUP = NCHUNK // GROUP           # 4
WIN_BASE = (0, 32, 96, 128)        # static slot-window base per group
F32 = mybir.dt.float32
BF16 = mybir.dt.bfloat16
NPBF16 = ml_dtypes.bfloat16

_compiled = {}  # variant -> (nc, out_name)


def host_scan(alphas: np.ndarray) -> tuple[np.ndarray, ...]:
    """Replicate the reference's sequential f32 scan exactly.

    Returns slotA, slotB (int label indices) and wA, wB (f32 weights),
    each [B, T]:  out[b, l] = sum_t (slotA==l)*wA*h_t + (slotB==l)*wB*h_t.
    """
    Bn, Tn = alphas.shape
    one = np.float32(1.0)
    thr = np.float32(0.95)
    integrate = np.zeros(Bn, np.float32)
    fire_all = np.zeros((Bn, Tn), bool)
    cur_all = np.empty((Bn, Tn), np.float32)
    rem_all = np.empty((Bn, Tn), np.float32)
    for t in range(Tn):
        at = alphas[:, t]
        dist = one - integrate
        integrate = integrate + at
        fire = integrate > thr
        integrate = np.where(fire, integrate - one, integrate)
        cur = np.where(fire, dist, at)
        fire_all[:, t] = fire
        cur_all[:, t] = cur
        rem_all[:, t] = at - cur

    k_t = np.cumsum(fire_all, axis=1)        # fires up to and including t
    n_before = k_t - fire_all                # fires strictly before t
    total = k_t[:, -1:]
    slotA = np.minimum(n_before, L - 1).astype(np.int64)
    slotB = np.minimum(k_t, L - 1).astype(np.int64)
    wA = np.where(n_before < total, cur_all, np.float32(0.0))
    wB = np.where(k_t < total, rem_all, np.float32(0.0))
    return slotA, slotB, wA, wB


def _window_ok(slotA, slotB, wA, wB) -> bool:
    """Every nonzero contribution must land inside its chunk-group window."""
    for g in range(NGROUP):
        base = WIN_BASE[g]
        sl = slice(g * GROUP * TCHUNK, (g + 1) * GROUP * TCHUNK)
        for s, w in ((slotA[:, sl], wA[:, sl]), (slotB[:, sl], wB[:, sl])):
            m = w != 0
            if m.any():
                v = s[m]
                if v.min() < base or v.max() > base + 127:
                    return False
    return True


def _expand_w(slotA, slotB, wA, wB, nrows: int) -> np.ndarray:
    """Dense weight tiles [B, 128(t within chunk), NCHUNK*nrows] bf16.

    Column c*nrows + r of partition p holds the weight of timestep
    t = c*128 + p into PSUM row r (row = slot mod nrows)."""
    rA = (slotA % nrows).astype(np.int64)
    rB = (slotB % nrows).astype(np.int64)
    w = np.zeros((B, T, nrows), np.float32)
    bt = np.arange(B * T)
    np.add.at(w.reshape(-1, nrows), (bt, rA.reshape(-1)), wA.reshape(-1))
    np.add.at(w.reshape(-1, nrows), (bt, rB.reshape(-1)), wB.reshape(-1))
    # [B, T, nrows] -> [B, NCHUNK, 128, nrows] -> [B, 128, NCHUNK, nrows]
    w = w.reshape(B, NCHUNK, TCHUNK, nrows).transpose(0, 2, 1, 3)
    return np.ascontiguousarray(w.reshape(B, TCHUNK, NCHUNK * nrows)).astype(NPBF16)


# Narrow-W layout: each chunk's slots span only ~26 rows (slot tracks
# 0.125*t), so many stationary tiles can be 64 rows wide instead of 128.
# HW: a matmul's PSUM write may only start at partition 0 or 64 for a
# 64-row tile, so narrow chunks are those whose expected rows sit inside
# [0,64) or [64,128); chunks whose range crosses row 64 (c=3, c=11), the
# wrap chunk (c=7), and each group's start=True chunk (must zero the whole
# window) stay full.  Host verifies every nonzero weight fits, else falls
# back to the uniform layout.
NARROW_BASE = {1: 0, 2: 0, 9: 0, 10: 0, 5: 64, 6: 64, 13: 64, 14: 64, 15: 64}
NARROW_W = 64


def _narrow_layout():
    offs, widths, bases, off = [], [], [], 0
    for c in range(NCHUNK):
        if c in NARROW_BASE:
            w_, r_ = NARROW_W, NARROW_BASE[c]
        else:
            w_, r_ = TCHUNK, 0
        offs.append(off); widths.append(w_); bases.append(r_)
        off += w_
    return offs, widths, bases, off


W_OFFS, W_WIDTHS, W_BASES, W_COLS = _narrow_layout()  # W_COLS = 992


def _expand_w_narrow(slotA, slotB, wA, wB):
    """Variable-width weight tiles [B, 128, W_COLS] bf16, or None if some
    contribution falls outside its chunk's narrow row range."""
    dense = _expand_w(slotA, slotB, wA, wB, TCHUNK)  # [B, 128, NCHUNK*128]
    dense = dense.reshape(B, TCHUNK, NCHUNK, TCHUNK)
    parts = []
    for c in range(NCHUNK):
        r, w_ = W_BASES[c], W_WIDTHS[c]
        tile_c = dense[:, :, c, :]
        if w_ != TCHUNK:
            inside = tile_c[:, :, r:r + w_]
            if np.count_nonzero(tile_c) != np.count_nonzero(inside):
                return None
            tile_c = inside
        parts.append(tile_c)
    return np.ascontiguousarray(np.concatenate(parts, axis=2))


def _pack_hidden(hidden: np.ndarray) -> np.ndarray:
    """[B, 128(t within chunk), NCHUNK*H] bf16: col c*H + h of partition p
    holds hidden[b, c*128 + p, h]."""
    hp = hidden.reshape(B, NCHUNK, TCHUNK, H).transpose(0, 2, 1, 3)
    return np.ascontiguousarray(hp.reshape(B, TCHUNK, NCHUNK * H)).astype(NPBF16)


def build_program_windowed(narrow: bool = False):
    """Fast path: one bf16 matmul per chunk into a static 128-slot window.
    With narrow=True the stationary tiles use the variable-width layout."""
    nc = bacc.Bacc("TRN2", target_bir_lowering=False, debug=False,
                   enable_partition_id=False)

    wcols = W_COLS if narrow else NCHUNK * TCHUNK
    hid_d = nc.dram_tensor("hidp", [B_PER_CORE, TCHUNK, NCHUNK * H], BF16,
                           kind="ExternalInput")
    w_d = nc.dram_tensor("wp", [B_PER_CORE, TCHUNK, wcols], BF16,
                         kind="ExternalInput")
    out_d = nc.dram_tensor("out", [B_PER_CORE, L, H], BF16, kind="ExternalOutput")

    GW = GROUP * TCHUNK   # w cols per group (512)
    GH = GROUP * H        # hidden cols per group (2048)

    with tile.TileContext(nc) as tc:
        with (
            tc.tile_pool(name="hid", bufs=16) as hidp,
            tc.tile_pool(name="wts", bufs=4) as wpool,
            tc.tile_pool(name="outp", bufs=2) as outp,
            tc.tile_pool(name="psum", bufs=8, space="PSUM") as psump,
        ):
            # Issue every load up front (10.4MB fits SBUF).  The two HWDGE
            # queues share ~380GB/s of per-core HBM bandwidth (a gpsimd
            # software-DGE third lane just steals from the same pool and
            # measures slower), so balance bytes: sync gets hidden groups
            # 0-1 + W for even batches (5.19MB), scalar gets hidden groups
            # 2-3 + W for odd batches (5.19MB); output writes alternate.
            hts, wts = {}, {}
            for i in range(B_PER_CORE):
                wt = wpool.tile([TCHUNK, wcols], BF16)
                weng = nc.sync if i % 2 == 0 else nc.scalar
                weng.dma_start(wt[:], w_d[i])
                wts[i] = wt
                for g in range(NGROUP):
                    ht = hidp.tile([TCHUNK, GH], BF16)
                    src = hid_d[i, :, g * GH:(g + 1) * GH]
                    eng = nc.sync if g < 2 else nc.scalar
                    if i == 0 and g in (0, 2):
                        # halve the queue-head loads so the first matmuls
                        # start earlier; finer splits shrink the descriptor
                        # size (1KB/partition) and measurably sag the HBM
                        # stream, so keep everything else at 4KB/partition
                        h2 = GH // 2
                        eng.dma_start(ht[:, 0:h2],
                                      hid_d[i, :, g * GH:g * GH + h2])
                        eng.dma_start(ht[:, h2:GH],
                                      hid_d[i, :, g * GH + h2:(g + 1) * GH])
                    else:
                        eng.dma_start(ht[:], src)
                    hts[i, g] = ht

            for i in range(B_PER_CORE):
                wt = wts[i]
                # separate tiles per output half: a DMA read waits on ALL
                # writers of its tile, so a shared [128, 2, H] tile made the
                # half-0 write falsely depend on half-1's combine
                ob0 = outp.tile([TCHUNK, H], BF16, tag="ob0")
                ob1 = outp.tile([TCHUNK, H], BF16, tag="ob1")
                ps = []
                for g in range(NGROUP):
                    ht = hts[i, g]
                    psg = psump.tile([TCHUNK, H], F32)
                    ps.append(psg)
                    for cc in range(GROUP):
                        c = g * GROUP + cc
                        if narrow:
                            off, wid, base = W_OFFS[c], W_WIDTHS[c], W_BASES[c]
                        else:
                            off, wid, base = c * TCHUNK, TCHUNK, 0
                        nc.tensor.matmul(
                            psg[base:base + wid, :], wt[:, off:off + wid],
                            ht[:, cc * H:(cc + 1) * H],
                            start=(cc == 0), stop=(cc == GROUP - 1),
                        )
                    # Combine overlapping windows as soon as deps allow
                    # (slot coverage: g0 0..127, g1 32..159, g2 96..223,
                    # g3 128..255; PSUM row = slot mod 128 so every slice
                    # is partition-aligned).  The ACT copy downcasts into
                    # the bf16 out tile; DVE accumulates overlap rows in
                    # place, so every add has exactly one PSUM operand and
                    # no staging tiles are needed (HW rules: one PSUM input
                    # per tensor_tensor; partition ranges starting at 32
                    # span max 32, at 64 max 64).
                    if g == 2:
                        nc.scalar.copy(ob0[:], ps[0][:])
                        nc.vector.tensor_add(
                            ob0[32:64, :], ob0[32:64, :], ps[1][32:64, :]
                        )
                        nc.vector.tensor_add(
                            ob0[64:128, :], ob0[64:128, :], ps[1][64:128, :]
                        )
                        nc.vector.tensor_add(
                            ob0[96:128, :], ob0[96:128, :], ps[2][96:128, :]
                        )
                        oeng0 = nc.gpsimd if i < 2 else nc.sync
                        oeng0.dma_start(out_d[i, 0:128, :], ob0[:])
                nc.scalar.copy(ob1[:], ps[3][:])
                nc.vector.tensor_add(ob1[0:96, :], ob1[0:96, :], ps[2][0:96, :])
                nc.vector.tensor_add(ob1[0:32, :], ob1[0:32, :], ps[1][0:32, :])
                oeng1 = nc.gpsimd if i < 2 else nc.scalar
                oeng1.dma_start(out_d[i, 128:256, :], ob1[:])

    nc.compile()
    return nc, out_d.name


def build_program_generic():
    """Fallback: full-width weights, two matmuls per chunk."""
    nc = bacc.Bacc("TRN2", target_bir_lowering=False, debug=False)

    hid_d = nc.dram_tensor("hidp", [B_PER_CORE, TCHUNK, NCHUNK * H], BF16,
                           kind="ExternalInput")
    w_d = nc.dram_tensor("wp", [B_PER_CORE, TCHUNK, NCHUNK * L], BF16,
                         kind="ExternalInput")
    out_d = nc.dram_tensor("out", [B_PER_CORE, L, H], BF16, kind="ExternalOutput")

    with tile.TileContext(nc) as tc:
        with (
            tc.tile_pool(name="hid", bufs=4) as hidp,
            tc.tile_pool(name="wts", bufs=4) as wpool,
            tc.tile_pool(name="outp", bufs=2) as outp,
            tc.tile_pool(name="psum", bufs=4, space="PSUM") as psump,
        ):
            for i in range(B_PER_CORE):
                ps0 = psump.tile([TCHUNK, H], F32)
                ps1 = psump.tile([TCHUNK, H], F32)
                for c in range(NCHUNK):
                    ht = hidp.tile([TCHUNK, H], BF16)
                    nc.sync.dma_start(ht[:], hid_d[i, :, c * H:(c + 1) * H])
                    wt = wpool.tile([TCHUNK, L], BF16)
                    nc.scalar.dma_start(wt[:], w_d[i, :, c * L:(c + 1) * L])
                    nc.tensor.matmul(
                        ps0[:], wt[:, 0:128], ht[:],
                        start=(c == 0), stop=(c == NCHUNK - 1),
                    )
                    nc.tensor.matmul(
                        ps1[:], wt[:, 128:256], ht[:],
                        start=(c == 0), stop=(c == NCHUNK - 1),
                    )
                o0 = outp.tile([128, H], BF16, tag="o0")
                nc.scalar.copy(o0[:], ps0[:])
                o1 = outp.tile([128, H], BF16, tag="o1")
                nc.scalar.copy(o1[:], ps1[:])
                nc.sync.dma_start(out_d[i, 0:128, :], o0[:])
                nc.sync.dma_start(out_d[i, 128:256, :], o1[:])

    nc.compile()
    return nc, out_d.name


def _get_compiled(variant: str):
    if variant not in _compiled:
        if variant == "narrow":
            _compiled[variant] = build_program_windowed(narrow=True)
        elif variant == "windowed":
            _compiled[variant] = build_program_windowed()
        else:
            _compiled[variant] = build_program_generic()
    return _compiled[variant]


def prepare(hidden: np.ndarray, alphas: np.ndarray):
    """Host scan + input packing. Returns (variant, in_maps)."""
    slotA, slotB, wA, wB = host_scan(alphas)
    w = None
    if _window_ok(slotA, slotB, wA, wB):
        w = _expand_w_narrow(slotA, slotB, wA, wB)
        if w is not None:
            variant = "narrow"
        else:
            variant = "windowed"
            w = _expand_w(slotA, slotB, wA, wB, TCHUNK)
    else:
        variant = "generic"
        w = _expand_w(slotA, slotB, wA, wB, L)
    hidp = _pack_hidden(hidden)
    in_maps = []
    for j in range(N_CORES):
        sl = slice(j * B_PER_CORE, (j + 1) * B_PER_CORE)
        in_maps.append({"hidp": hidp[sl], "wp": w[sl]})
    return variant, in_maps


def run_sharded(hidden: np.ndarray, alphas: np.ndarray, trace: bool = False, **kw):
    """Run the SPMD kernel; returns (out [B,L,H] f32, BassKernelResults)."""
    variant, in_maps = prepare(hidden, alphas)
    nc, out_name = _get_compiled(variant)
    res = run_bass_kernel_spmd(nc, in_maps, list(range(N_CORES)), trace=trace, **kw)
    out = np.concatenate([r[out_name] for r in res.results], axis=0)
    return out.astype(np.float32), res


def kernel(hidden, alphas, num_labels=L) -> np.ndarray:
    hidden = np.asarray(hidden, dtype=np.float32)
    alphas = np.asarray(alphas, dtype=np.float32)
    assert hidden.shape == (B, T, H) and alphas.shape == (B, T)
    assert int(num_labels) == L
    out, _ = run_sharded(hidden, alphas)
    return out
